# revision 2
# baseline (speedup 1.0000x reference)
"""Trainium2 Bass kernel for nn_DenoisingLocal_Global_ConvNN_2D.

Network (per sample):
  conv3x3(3->16, pad 1) + ReLU
  -> pixel_unshuffle(2): m2 (64, 1024)  [tokens = 32x32 grid]
  -> kNN layer A: all-pairs dist on m2, top-9 (self always rank 0),
     y2 = W2_0 @ m2 + sum_{k=1..8} W2_k @ m2[:, idx_k] + b2, ReLU -> m3 (128, 1024)
  -> kNN layer B on m3: y3 = W3_0 @ m3 + sum_k W3_k @ m3[:, idx_k] + b3 (12, 1024)
  -> pixel_shuffle(2) -> (3, 64, 64)
(pixel_shuffle∘pixel_unshuffle between the two kNN layers is the identity, so
 both kNN layers run in the same 1024-token space.)

Distance trick: ranking of -dist within a row equals ranking of
A[n,j] = 2*m^T m - nsq[j]; computed as one matmul with an extra contraction row
(lhsT row = 1, rhs row = -nsq).  Self is suppressed by adding -3e38 to the
diagonal; then hardware max8/max_index give ranks 1..8 directly.

Neighbor gathers use dma_gather (wrapped-16 int16 index tables); the tables are
built with two small DRAM shuffle DMAs per layer.

Sharding: pure data parallelism, 8 samples per NeuronCore x 8 cores.
"""
import sys

for _p in ('/opt/trn_rl_repo',):
    if _p not in sys.path:
        sys.path.insert(0, _p)

import numpy as np
from contextlib import ExitStack

import concourse.bass as bass
import concourse.tile as tile
from concourse import bacc, mybir
from concourse import bass_utils

F32 = mybir.dt.float32
U16 = mybir.dt.uint16
I16 = mybir.dt.int16
AF = mybir.ActivationFunctionType

N_CORES = 8
SAMPLES = 8          # samples per core
NEG_BIG = -3.0e38

# matmul dtype knob (float32 = exact 4 cyc/row; float32r = fast, reduced prec)
DIST_DT = mybir.dt.float32r
CONV_DT = mybir.dt.float32r


def _bc(ap, dt):
    if ap.dtype == dt:
        return ap
    return ap.bitcast(dt)


# ----------------------------------------------------------------------------
# host-side input preparation (numpy)
# ----------------------------------------------------------------------------

def build_consts(w1, b1, w2, b2, w3, b3):
    w1 = np.asarray(w1, np.float32).reshape(16, 3, 3, 3)
    b1 = np.asarray(b1, np.float32)
    w2 = np.asarray(w2, np.float32).reshape(128, 64, 9)
    b2 = np.asarray(b2, np.float32)
    w3 = np.asarray(w3, np.float32).reshape(12, 128, 9)
    b3 = np.asarray(b3, np.float32)

    # conv1 lhsT: 4 phases, K=28 (27 taps + bias row), M=64 (16 ch x 4 phases)
    c1 = np.zeros((4, 28, 64), np.float32)
    for q in range(4):
        for dy in range(3):
            for dx in range(3):
                c1[q, np.arange(3)[:, None] * 9 + dy * 3 + dx,
                   np.arange(16)[None, :] * 4 + q] = w1[:, :, dy, dx].T
        c1[q, 27, np.arange(16) * 4 + q] = b1
    c1 = np.ascontiguousarray(c1.transpose(1, 0, 2).reshape(28, 256))

    # W2 neighbor pair K-tiles: (128, 4*128); pair P cols [128P:128P+128]
    # K-tile rows [64h + c] hold tap (2P + h + 1)
    w2pairs = np.zeros((128, 512), np.float32)
    for P in range(4):
        for h in range(2):
            w2pairs[64 * h:64 * h + 64, 128 * P:128 * P + 128] = \
                w2[:, :, 2 * P + h + 1].T
    w2self = np.zeros((65, 128), np.float32)
    w2self[:64] = w2[:, :, 0].T
    w2self[64] = b2

    # output-channel permutation: co = ch*4+q -> co' = q*3+ch so each
    # pixel_shuffle phase q reads contiguous partitions [3q:3q+3]
    perm = np.zeros(12, np.int64)
    for ch in range(3):
        for q in range(4):
            perm[q * 3 + ch] = ch * 4 + q
    w3 = w3[perm]
    b3 = b3[perm]

    # W3 zcat lhsT (128, 96): col 12k+co'
    w3zcat = np.zeros((128, 96), np.float32)
    for k in range(8):
        w3zcat[:, 12 * k:12 * k + 12] = w3[:, :, k + 1].T
    w3self = np.ascontiguousarray(w3[:, :, 0].T)          # (128, 12)
    b3col = np.ascontiguousarray(b3[:, None])             # (12, 1)

    ident = np.eye(128, dtype=np.float32)
    diagneg = np.zeros((128, 128), np.float32)
    np.fill_diagonal(diagneg, NEG_BIG)

    return dict(c1=c1, w2pairs=w2pairs, w2self=w2self,
                w3zcat=w3zcat, w3self=w3self, b3col=b3col, ident=ident,
                diagneg=diagneg)


def build_p27(x_shard):
    """Per-phase im2col for conv1: (S, 4, 28, 1024).
    p27[s, q=(sy,sx), 9ci+3dy+dx, 32y+x] = xpad[s, ci, 2y+sy+dy, 2x+sx+dx];
    row 27 = 1.0 (bias)."""
    S = x_shard.shape[0]
    xp = np.zeros((S, 3, 66, 66), np.float32)
    xp[:, :, 1:65, 1:65] = x_shard
    p27 = np.ones((S, 4, 28, 1024), np.float32)
    for q in range(4):
        sy, sx = q >> 1, q & 1
        for ci in range(3):
            for dy in range(3):
                for dx in range(3):
                    v = xp[:, ci, sy + dy:sy + dy + 64:2, sx + dx:sx + dx + 64:2]
                    p27[:, q, ci * 9 + dy * 3 + dx, :] = v.reshape(S, 1024)
    return p27


# ----------------------------------------------------------------------------
# device program
# ----------------------------------------------------------------------------

def _ap(base_ap, offset, dims):
    return bass.AP(base_ap.tensor, offset, [list(d) for d in dims])


def _emit_wrapped_idx(nc, dp, sp, idxv, tag):
    """idxv: SBUF (128, 64) u16 AP, value for slot s = blk*128 + p at [p, blk].
    Produces the wrapped dma_gather table (128, 512) u16:
      W[16r + b, 8*blk + a] = idxv[16a + b, blk]   (replicated over r)
    via two DRAM hops (all DMA APs <= 3 dims)."""
    da = dp.tile([8192], U16, tag=tag + "a")
    # plain dump: da flat = p*64 + blk = a*1024 + b*64 + blk (p = 16a + b)
    nc.sync.dma_start(da[:], idxv)
    db = dp.tile([8192], U16, tag=tag + "b")
    # shuffle: dst flat = b*512 + blk*8 + a <- src a*1024 + b*64 + blk
    nc.sync.dma_start(
        _ap(db[:], 0, [[512, 16], [8, 64], [1, 8]]),
        _ap(da[:], 0, [[64, 16], [1, 64], [1024, 8]]))
    w = sp.tile([128, 512], U16, tag=tag + "w")
    # replicate into (16r + b, c) <- src b*512 + c
    nc.sync.dma_start(
        w[:], _ap(db[:], 0, [[0, 8], [512, 16], [1, 512]]))
    return w


def build_program(nc, samples=SAMPLES, tap=None, stage=99, repeat=1):
    p27_d = nc.dram_tensor("p27", (samples, 4, 28, 1024), F32, kind="ExternalInput").ap()
    c1_d = nc.dram_tensor("c1", (28, 256), F32, kind="ExternalInput").ap()
    w2p_d = nc.dram_tensor("w2pairs", (128, 512), F32, kind="ExternalInput").ap()
    w2s_d = nc.dram_tensor("w2self", (65, 128), F32, kind="ExternalInput").ap()
    w3z_d = nc.dram_tensor("w3zcat", (128, 96), F32, kind="ExternalInput").ap()
    w3s_d = nc.dram_tensor("w3self", (128, 12), F32, kind="ExternalInput").ap()
    b3_d = nc.dram_tensor("b3col", (12, 1), F32, kind="ExternalInput").ap()
    id_d = nc.dram_tensor("ident", (128, 128), F32, kind="ExternalInput").ap()
    dg_d = nc.dram_tensor("diagneg", (128, 128), F32, kind="ExternalInput").ap()
    out_d = nc.dram_tensor("out", (samples, 12, 1024), F32, kind="ExternalOutput").ap()

    if tap is None:
        def tap(name, t):
            pass

    with tile.TileContext(nc) as tc, ExitStack() as ctx:
        cp = ctx.enter_context(tc.tile_pool(name="consts", bufs=1))
        sp = ctx.enter_context(tc.tile_pool(name="sb", bufs=2))
        bp = ctx.enter_context(tc.tile_pool(name="big", bufs=1))
        gp = ctx.enter_context(tc.tile_pool(name="gm", bufs=5))
        pp = ctx.enter_context(tc.tile_pool(name="ps", bufs=4, space="PSUM"))
        dp = ctx.enter_context(tc.tile_pool(name="dram", bufs=2, space="DRAM"))

        c1 = cp.tile([28, 256], F32); nc.sync.dma_start(c1[:], c1_d)
        w2p = cp.tile([128, 512], F32); nc.sync.dma_start(w2p[:], w2p_d)
        w2s = cp.tile([65, 128], F32); nc.sync.dma_start(w2s[:], w2s_d)
        w3z = cp.tile([128, 96], F32); nc.sync.dma_start(w3z[:], w3z_d)
        w3s = cp.tile([128, 12], F32); nc.sync.dma_start(w3s[:], w3s_d)
        b3c = cp.tile([12, 1], F32); nc.sync.dma_start(b3c[:], b3_d)
        ident = cp.tile([128, 128], F32); nc.sync.dma_start(ident[:], id_d)
        diag = cp.tile([128, 128], F32); nc.sync.dma_start(diag[:], dg_d)
        ones64 = cp.tile([64, 1], F32); nc.gpsimd.memset(ones64[:], 1.0)
        ones128 = cp.tile([128, 1], F32); nc.gpsimd.memset(ones128[:], 1.0)
        onesr = cp.tile([1, 128], F32); nc.gpsimd.memset(onesr[:], 1.0)

        for s in [ss for _ in range(repeat) for ss in range(samples)]:
            # ================= conv1 =================
            p27 = bp.tile([28, 4096], F32, tag="p27")
            nc.sync.dma_start(p27[:].rearrange("p (q n) -> p q n", q=4),
                              p27_d[s].rearrange("q p n -> p q n"))
            H = pp.tile([128, 1024], F32, tag="ps")
            for cch in range(2):
                for q in range(4):
                    rhs = p27[:, 1024 * q + 512 * cch:1024 * q + 512 * cch + 512]
                    nc.tensor.matmul(H[:64, 512 * cch:512 * cch + 512],
                                     _bc(c1[:, 64 * q:64 * q + 64], CONV_DT),
                                     _bc(rhs, CONV_DT), start=(q == 0), stop=(q == 3))
            m2l = sp.tile([65, 1024], F32, tag="m2l")
            nc.scalar.activation(m2l[:64, :], H[:64, :], AF.Relu)
            nc.gpsimd.memset(m2l[64:65, :], 1.0)
            m2r = sp.tile([65, 1024], F32, tag="m2r")
            nc.scalar.activation(m2r[:64, :], H[:64, :], AF.Relu, scale=2.0)
            msq = sp.tile([64, 1024], F32, tag="msq")
            nc.scalar.activation(msq[:], m2l[:64, :], AF.Square)
            nsqp = pp.tile([128, 1024], F32, tag="ps")
            for cch in range(2):
                nc.tensor.matmul(nsqp[:1, 512 * cch:512 * cch + 512], ones64[:],
                                 msq[:, 512 * cch:512 * cch + 512],
                                 start=True, stop=True)
            nc.scalar.activation(m2r[64:65, :], nsqp[:1, :], AF.Copy, scale=-1.0)
            tap("m2l_%d" % s, m2l[:])
            tap("m2r_%d" % s, m2r[:])

            # mT2 to DRAM for the neighbor gather
            ttp = pp.tile([128, 1024], F32, tag="ps")
            for t in range(8):
                nc.tensor.matmul(ttp[:, 64 * t:64 * t + 64],
                                 m2l[:64, 128 * t:128 * t + 128],
                                 ident[:64, :64], is_transpose=True,
                                 start=True, stop=True)
            mt2sb = sp.tile([128, 512], F32, tag="mt2")
            nc.scalar.activation(mt2sb[:], ttp[:, :512], AF.Copy)
            mt2_dram = dp.tile([1024, 64], F32, tag="mt2d")
            nc.sync.dma_start(
                mt2_dram[:].rearrange("(t p) c -> p t c", p=128),
                mt2sb[:].rearrange("p (t c) -> p t c", t=8))
            tap("mt2_%d" % s, mt2_dram[:])
            if stage < 2:
                continue

            # ================= layer A kNN =================
            idxn = sp.tile([128, 8, 8], U16, tag="idxn")   # [p][t][k]
            vals2 = sp.tile([128, 8, 8], F32, tag="vals2")
            for t in range(8):
                A = pp.tile([128, 1024], F32, tag="ps")
                for cch in range(2):
                    nc.tensor.matmul(A[:, 512 * cch:512 * cch + 512],
                                     _bc(m2l[:, 128 * t:128 * t + 128], DIST_DT),
                                     _bc(m2r[:, 512 * cch:512 * cch + 512], DIST_DT),
                                     start=True, stop=True)
                nc.vector.tensor_add(A[:, 128 * t:128 * t + 128],
                                     A[:, 128 * t:128 * t + 128], diag[:])
                nc.vector.max(vals2[:, t, :], A[:])
                nc.vector.max_index(idxn[:, t, :], vals2[:, t, :], A[:])
            tap("idxn_%d" % s, idxn[:])
            if stage < 3:
                continue

            # permute [p][t][k] -> [p][blk] with blk = 16P + 2t + h, k = 2P+h
            idxv2 = sp.tile([128, 64], U16, tag="idxv2")
            nc.vector.tensor_copy(
                idxv2[:].rearrange("p (P t h) -> p P t h", P=4, t=8),
                idxn[:].rearrange("p t (P h) -> p P t h", P=4))
            w2idx = _emit_wrapped_idx(nc, dp, sp, idxv2[:], "i2")
            g2 = bp.tile([128, 4, 8, 2, 64], F32, tag="g2")  # [p][P][t][h][c]
            nc.gpsimd.dma_gather(
                g2[:].rearrange("p P t h c -> p (P t h) c"),
                mt2_dram[:], _bc(w2idx[:], I16), 8192, 8192, 64,
                single_packet=False)
            tap("g2_%d" % s, g2[:].rearrange("p P t h c -> p (P t h) c"))

            # transpose pairs -> feature-major K-tiles, then conv
            y2 = pp.tile([128, 1024], F32, tag="ps")
            gmats = []
            for P in range(4):
                gtp = pp.tile([128, 1024], F32, tag="ps")
                for t in range(8):
                    nc.tensor.matmul(
                        gtp[:, 128 * t:128 * t + 128],
                        g2[:, P, t, :, :].rearrange("p h c -> p (h c)"),
                        ident[:], is_transpose=True, start=True, stop=True)
                gm = gp.tile([128, 1024], F32, tag="gmat")
                nc.scalar.activation(gm[:], gtp[:], AF.Copy)
                gmats.append(gm)
            for cch in range(2):
                sl = slice(512 * cch, 512 * cch + 512)
                for P in range(4):
                    nc.tensor.matmul(y2[:, sl],
                                     _bc(w2p[:, 128 * P:128 * P + 128], CONV_DT),
                                     _bc(gmats[P][:, sl], CONV_DT),
                                     start=(P == 0), stop=False)
                nc.tensor.matmul(y2[:, sl], _bc(w2s[:], CONV_DT),
                                 _bc(m2l[:, sl], CONV_DT), start=False, stop=True)

            if stage < 4:
                continue
            # ================= layer B prep =================
            m3 = sp.tile([128, 1024], F32, tag="m3")
            nc.scalar.activation(m3[:], y2[:], AF.Relu)
            tap("m3_%d" % s, m3[:])
            m3r = sp.tile([128, 1024], F32, tag="m3r")
            nc.scalar.activation(m3r[:], y2[:], AF.Relu, scale=2.0)
            msq3 = sp.tile([128, 1024], F32, tag="msq3")
            nc.scalar.activation(msq3[:], m3[:], AF.Square)
            nsq3p = pp.tile([128, 1024], F32, tag="ps")
            for cch in range(2):
                nc.tensor.matmul(nsq3p[:1, 512 * cch:512 * cch + 512], ones128[:],
                                 msq3[:, 512 * cch:512 * cch + 512],
                                 start=True, stop=True)
            nsq3n = sp.tile([1, 1024], F32, tag="nsq3")
            nc.scalar.activation(nsq3n[:], nsq3p[:1, :], AF.Copy, scale=-1.0)

            # Zcat (96, 1024) = taps 1..8 of W3 @ m3; ZT (1024, 128-pad) to DRAM
            zcp = pp.tile([128, 1024], F32, tag="ps")
            for cch in range(2):
                nc.tensor.matmul(zcp[:96, 512 * cch:512 * cch + 512],
                                 _bc(w3z[:], CONV_DT),
                                 _bc(m3[:, 512 * cch:512 * cch + 512], CONV_DT),
                                 start=True, stop=True)
            zcs = sp.tile([96, 1024], F32, tag="zcs")
            nc.scalar.activation(zcs[:], zcp[:96, :], AF.Copy)
            # rows padded to 128 f32 (512B) for dma_gather alignment
            ztsb = sp.tile([128, 8, 128], F32, tag="ztsb")
            nc.gpsimd.memset(ztsb[:, :, 96:], 0.0)
            for g in range(2):
                ztp = pp.tile([128, 1024], F32, tag="ps")
                for tt in range(4):
                    t = 4 * g + tt
                    nc.tensor.matmul(ztp[:, 96 * tt:96 * tt + 96],
                                     zcs[:, 128 * t:128 * t + 128],
                                     ident[:96, :96], is_transpose=True,
                                     start=True, stop=True)
                nc.scalar.activation(
                    _ap(ztsb[:], 4 * g * 128, [[1024, 128], [128, 4], [1, 96]]),
                    ztp[:, :384], AF.Copy)
            zt3_dram = dp.tile([1024, 128], F32, tag="zt3d")
            nc.sync.dma_start(
                zt3_dram[:].rearrange("(t p) c -> p t c", p=128),
                ztsb[:])
            tap("zt3_%d" % s, zt3_dram[:])

            if stage < 5:
                continue
            # ================= layer B kNN =================
            idx3 = sp.tile([128, 8, 8], U16, tag="idx3")   # [p][t][k]; blk = 8t+k
            vals3 = sp.tile([128, 8, 8], F32, tag="vals3")
            for t in range(8):
                A = pp.tile([128, 1024], F32, tag="ps")
                for cch in range(2):
                    sl = slice(512 * cch, 512 * cch + 512)
                    nc.tensor.matmul(A[:, sl],
                                     _bc(m3[:, 128 * t:128 * t + 128], DIST_DT),
                                     _bc(m3r[:, sl], DIST_DT), start=True, stop=False)
                    nc.tensor.matmul(A[:, sl], _bc(onesr[:], DIST_DT),
                                     _bc(nsq3n[:, sl], DIST_DT),
                                     start=False, stop=True)
                nc.vector.tensor_add(A[:, 128 * t:128 * t + 128],
                                     A[:, 128 * t:128 * t + 128], diag[:])
                nc.vector.max(vals3[:, t, :], A[:])
                nc.vector.max_index(idx3[:, t, :], vals3[:, t, :], A[:])
            tap("idx3_%d" % s, idx3[:])

            if stage < 6:
                continue
            w3idx = _emit_wrapped_idx(nc, dp, sp,
                                      idx3[:].rearrange("p t k -> p (t k)"), "i3")
            g3 = bp.tile([128, 64, 128], F32, tag="g3")    # [p][blk=8t+k][128]
            nc.gpsimd.dma_gather(
                g3[:], zt3_dram[:], _bc(w3idx[:], I16), 8192, 8192, 128,
                single_packet=False)
            tap("g3_%d" % s, g3[:])

            if stage < 7:
                continue
            # r3[p, t, c] = sum_k g3[p, 8t+k, 12k + c]
            r3 = sp.tile([128, 8, 12], F32, tag="r3")
            red_in = _ap(g3[:], 0, [[8192, 128], [1024, 8], [1, 12], [140, 8]])
            nc.vector.tensor_reduce(r3[:], red_in, axis=mybir.AxisListType.X,
                                    op=mybir.AluOpType.add)
            tap("r3_%d" % s, r3[:])

            # y3 = W3_0 @ m3 + bias + neighbor sums (transposed back)
            y3p = pp.tile([128, 1024], F32, tag="ps")
            for cch in range(2):
                nc.tensor.matmul(y3p[:12, 512 * cch:512 * cch + 512],
                                 _bc(w3s[:], CONV_DT),
                                 _bc(m3[:, 512 * cch:512 * cch + 512], CONV_DT),
                                 start=True, stop=False)
            for t in range(8):
                nc.tensor.matmul(y3p[:12, 128 * t:128 * t + 128],
                                 r3[:, t, :], ident[:], is_transpose=True,
                                 start=False, stop=(t % 4 == 3))
            y3sb = sp.tile([12, 1024], F32, tag="y3")
            nc.scalar.activation(y3sb[:], y3p[:12, :], AF.Identity, bias=b3c[:])
            tap("y3_%d" % s, y3sb[:])

            # contiguous store; pixel_shuffle happens on the host
            nc.sync.dma_start(out_d[s], y3sb[:])

    return nc


_CACHE = {}


def _get_compiled():
    if 'nc' not in _CACHE:
        nc = bacc.Bacc("TRN2", target_bir_lowering=False, debug=False,
                       num_devices=N_CORES)
        build_program(nc, SAMPLES)
        nc.compile()
        _CACHE['nc'] = nc
    return _CACHE['nc']


def make_in_maps(x, consts):
    in_maps = []
    for c in range(N_CORES):
        shard = np.ascontiguousarray(x[c * SAMPLES:(c + 1) * SAMPLES],
                                     dtype=np.float32)
        m = dict(consts)
        m['p27'] = build_p27(shard).astype(np.float32)
        in_maps.append(m)
    return in_maps


def kernel(x, conv1_w, conv1_b, conv2_w, conv2_b, conv3_w, conv3_b, **_ignored):
    x = np.asarray(x, np.float32)
    consts = build_consts(conv1_w, conv1_b, conv2_w, conv2_b, conv3_w, conv3_b)
    nc = _get_compiled()
    in_maps = make_in_maps(x, consts)
    res = bass_utils.run_bass_kernel_spmd(nc, in_maps, core_ids=list(range(N_CORES)))
    y3 = np.concatenate([res.results[c]['out'] for c in range(N_CORES)], axis=0)
    return shuffle_out(y3)


def shuffle_out(y3):
    """y3 (B, 12, 1024) with channel rows co' = q*3+ch -> (B, 3, 64, 64)."""
    B = y3.shape[0]
    y = y3.reshape(B, 4, 3, 32, 32)                # [b][q=(sy,sx)][ch][h][w]
    out = np.zeros((B, 3, 64, 64), np.float32)
    for q in range(4):
        sy, sx = q >> 1, q & 1
        out[:, :, sy::2, sx::2] = y[:, q]
    return out.astype(np.float32)


if __name__ == '__main__':
    nc = _get_compiled()
    print("compiled ok")



# revision 29
# speedup vs baseline: 2.6577x; 2.6577x over previous
"""Trainium2 Bass kernel for nn_DenoisingLocal_Global_ConvNN_2D.

Network (per sample):
  conv3x3(3->16, pad 1) + ReLU
  -> pixel_unshuffle(2): m2 (64, 1024)  [tokens = 32x32 grid]
  -> kNN layer A: all-pairs dist on m2, top-9 (self always rank 0),
     y2 = W2_0 @ m2 + sum_{k=1..8} W2_k @ m2[:, idx_k] + b2, ReLU -> m3 (128, 1024)
  -> kNN layer B on m3: y3 = W3_0 @ m3 + sum_k W3_k @ m3[:, idx_k] + b3 (12, 1024)
  -> pixel_shuffle(2) -> (3, 64, 64)

Precision strategy (the kNN ranking is ill-conditioned: a 4e-4 perturbation
of m2 already swaps ~1% of neighbor ranks, which cascades through both
layers):
  - conv1 runs as an fp16 hi/lo split (3 fp16 matmuls instead of one fp32
    matmul at 4 cyc/row) -> m2 exact to ~1e-7.
  - distance scores S = 2*m^T m - nsq_j run in full fp32 on the PE
    (layer B's -nsq_j rank-1 uses an fp16 hi/lo pair: fp32 accuracy at fp16
    matmul speed); top-8 = DVE max8/max_index on the fp32 scores.
  - layer A's conv feeds layer B's scores, so its neighbor gather moves
    fp16 hi/lo feature pairs (one 256B payload) and y2 accumulates
    whi@ghi + wlo@ghi + whi@glo (exact to ~1e-7).
  - layer B's conv output is terminal, so it runs plain fp16.

Neighbor gathers run straight from SBUF (dma_gather transpose mode): the
token-major source is built with DMA-XBAR transposes (dma_start
transpose=True, fp16), and the gather output is already feature-major, so
no PE transposes and no DRAM round trips for data.  Only the small wrapped-16
index tables go through DRAM (2 shuffle hops).

The program is emitted software-pipelined: each sample is split into 6
stages (load / conv1+prep / kNN-A / conv-A+prep-B / kNN-B / conv-B+store)
and waves emit stage k of sample w-k, so every engine's in-order queue
interleaves independent work from ~6 samples.  Distance scores are copied
PSUM->SBUF (Activation) so the top-k chain never holds PSUM slots.

Sharding: pure data parallelism, 8 samples per NeuronCore x 8 cores.
"""
import sys

for _p in ('/opt/trn_rl_repo',):
    if _p not in sys.path:
        sys.path.insert(0, _p)

import numpy as np
from contextlib import ExitStack

import concourse.bass as bass
import concourse.tile as tile
from concourse import bacc, mybir
from concourse import bass_utils

F32 = mybir.dt.float32
F16 = mybir.dt.float16
U16 = mybir.dt.uint16
I16 = mybir.dt.int16
AF = mybir.ActivationFunctionType

N_CORES = 8
SAMPLES = 8          # samples per core
NEG_BIG = -3.0e38    # diag suppression on the fp32 scores


def _split16(a):
    hi = a.astype(np.float16)
    lo = (a - hi.astype(np.float32)).astype(np.float16)
    return hi, lo


# ----------------------------------------------------------------------------
# host-side input preparation (numpy)
# ----------------------------------------------------------------------------

def build_consts(w1, b1, w2, b2, w3, b3):
    w1 = np.asarray(w1, np.float32).reshape(16, 3, 3, 3)
    b1 = np.asarray(b1, np.float32)
    w2 = np.asarray(w2, np.float32).reshape(128, 64, 9)
    b2 = np.asarray(b2, np.float32)
    w3 = np.asarray(w3, np.float32).reshape(12, 128, 9)
    b3 = np.asarray(b3, np.float32)

    # conv1 lhsT: 4 phases, K=28 (27 taps + bias row), M=64 (16 ch x 4 phases)
    c1 = np.zeros((4, 28, 64), np.float32)
    for q in range(4):
        for dy in range(3):
            for dx in range(3):
                c1[q, np.arange(3)[:, None] * 9 + dy * 3 + dx,
                   np.arange(16)[None, :] * 4 + q] = w1[:, :, dy, dx].T
        c1[q, 27, np.arange(16) * 4 + q] = b1
    c1 = np.ascontiguousarray(c1.transpose(1, 0, 2).reshape(28, 256))
    c1hi, c1lo = _split16(c1)

    # W2 rank K-tiles: (64, 8*128); rank kk cols [128kk:128kk+128] = tap kk+1
    w2r = np.zeros((64, 1024), np.float32)
    for kk in range(8):
        w2r[:, 128 * kk:128 * kk + 128] = w2[:, :, kk + 1].T
    w2self = np.zeros((65, 128), np.float32)
    w2self[:64] = w2[:, :, 0].T
    w2self[64] = b2
    w2rh, w2rl = _split16(w2r)
    # gather payloads stack [hi | lo] halves, so two base-0 K=128 matmuls
    # compute whi@ghi + whi@glo (w2A = [whi; whi]) and wlo@ghi (w2B = [wlo; 0])
    w2A = np.concatenate([w2rh, w2rh], axis=0).astype(np.float16)
    w2B = np.concatenate([w2rl, np.zeros((64, 1024), np.float32)],
                         axis=0).astype(np.float16)

    # output-channel permutation: co = ch*4+q -> co' = q*3+ch so each
    # pixel_shuffle phase q reads contiguous partitions [3q:3q+3]
    perm = np.zeros(12, np.int64)
    for ch in range(3):
        for q in range(4):
            perm[q * 3 + ch] = ch * 4 + q
    w3 = w3[perm]
    b3 = b3[perm]

    # W3 rank tiles: (128, 8*12); rank kk cols [12kk:12kk+12] = tap kk+1
    w3r = np.zeros((128, 96), np.float32)
    for kk in range(8):
        w3r[:, 12 * kk:12 * kk + 12] = w3[:, :, kk + 1].T
    w3self = np.ascontiguousarray(w3[:, :, 0].T)          # (128, 12)
    b3col = np.ascontiguousarray(b3[:, None])             # (12, 1)

    diagneg = np.zeros((128, 128), np.float32)
    np.fill_diagonal(diagneg, NEG_BIG)

    w2sh, w2sl = _split16(w2self)
    return dict(c1hi=c1hi, c1lo=c1lo,
                w2A=w2A, w2B=w2B, w2sh=w2sh, w2sl=w2sl,
                w3r=w3r.astype(np.float16),
                w3self=w3self.astype(np.float16), b3col=b3col,
                diagneg=diagneg)


def build_p27(x_shard):
    """Per-phase im2col for conv1, fp16 hi/lo split: 2x (S, 4, 28, 1024).
    p27[s, q=(sy,sx), 9ci+3dy+dx, 32y+x] = xpad[s, ci, 2y+sy+dy, 2x+sx+dx];
    row 27 = 1.0 (bias)."""
    S = x_shard.shape[0]
    xp = np.zeros((S, 3, 66, 66), np.float32)
    xp[:, :, 1:65, 1:65] = x_shard
    p27 = np.ones((S, 4, 28, 1024), np.float32)
    for q in range(4):
        sy, sx = q >> 1, q & 1
        for ci in range(3):
            for dy in range(3):
                for dx in range(3):
                    v = xp[:, ci, sy + dy:sy + dy + 64:2, sx + dx:sx + dx + 64:2]
                    p27[:, q, ci * 9 + dy * 3 + dx, :] = v.reshape(S, 1024)
    return _split16(p27)


# ----------------------------------------------------------------------------
# device program
# ----------------------------------------------------------------------------

def _ap(base_ap, offset, dims):
    return bass.AP(base_ap.tensor, offset, [list(d) for d in dims])


def _emit_wrapped_idx(nc, dp, sp, idxv, tag, nblk=32):
    """idxv: SBUF (128, nblk) u16 AP, value for slot s = blk*128 + p at
    [p, blk].  Produces the wrapped dma_gather table (128, 8*nblk) u16:
      W[16r + b, 8*blk + a] = idxv[16a + b, blk]   (replicated over r)
    via two DRAM hops (all DMA APs <= 3 dims)."""
    nc_ = nblk * 8
    da = dp.tile([128 * nblk], U16, tag=tag + "a")
    # plain dump: da flat = p*nblk + blk = a*16*nblk + b*nblk + blk (p = 16a+b)
    nc.sync.dma_start(da[:], idxv)
    db = dp.tile([128 * nblk], U16, tag=tag + "b")
    # shuffle: dst flat = b*8nblk + blk*8 + a <- src a*16nblk + b*nblk + blk
    nc.sync.dma_start(
        _ap(db[:], 0, [[nc_, 16], [8, nblk], [1, 8]]),
        _ap(da[:], 0, [[nblk, 16], [1, nblk], [16 * nblk, 8]]))
    w = sp.tile([128, nc_], U16, tag=tag + "w", bufs=4)
    # replicate into (16r + b, c) <- src b*8nblk + c
    nc.sync.dma_start(
        w[:], _ap(db[:], 0, [[0, 8], [nc_, 16], [1, nc_]]))
    return w


def build_program(nc, samples=SAMPLES, tap=None, stage=99, repeat=1):
    p27h_d = nc.dram_tensor("p27hi", (samples, 4, 28, 1024), F16, kind="ExternalInput").ap()
    p27l_d = nc.dram_tensor("p27lo", (samples, 4, 28, 1024), F16, kind="ExternalInput").ap()
    c1h_d = nc.dram_tensor("c1hi", (28, 256), F16, kind="ExternalInput").ap()
    c1l_d = nc.dram_tensor("c1lo", (28, 256), F16, kind="ExternalInput").ap()
    w2A_d = nc.dram_tensor("w2A", (128, 1024), F16, kind="ExternalInput").ap()
    w2B_d = nc.dram_tensor("w2B", (128, 1024), F16, kind="ExternalInput").ap()
    w2sh_d = nc.dram_tensor("w2sh", (65, 128), F16, kind="ExternalInput").ap()
    w2sl_d = nc.dram_tensor("w2sl", (65, 128), F16, kind="ExternalInput").ap()
    w3r_d = nc.dram_tensor("w3r", (128, 96), F16, kind="ExternalInput").ap()
    w3s_d = nc.dram_tensor("w3self", (128, 12), F16, kind="ExternalInput").ap()
    b3_d = nc.dram_tensor("b3col", (12, 1), F32, kind="ExternalInput").ap()
    dg_d = nc.dram_tensor("diagneg", (128, 128), F32, kind="ExternalInput").ap()
    out_d = nc.dram_tensor("out", (samples, 12, 1024), F32, kind="ExternalOutput").ap()

    if tap is None:
        def tap(name, t):
            pass

    with tile.TileContext(nc) as tc, ExitStack() as ctx:
        cp = ctx.enter_context(tc.tile_pool(name="consts", bufs=1))
        sp = ctx.enter_context(tc.tile_pool(name="sb", bufs=2))
        bp = ctx.enter_context(tc.tile_pool(name="big", bufs=2))
        pp = ctx.enter_context(tc.tile_pool(name="ps", bufs=2, space="PSUM"))
        dp = ctx.enter_context(tc.tile_pool(name="dram", bufs=2, space="DRAM"))

        c1h = cp.tile([28, 256], F16); nc.sync.dma_start(c1h[:], c1h_d)
        c1l = cp.tile([28, 256], F16); nc.sync.dma_start(c1l[:], c1l_d)
        w2A = cp.tile([128, 1024], F16); nc.sync.dma_start(w2A[:], w2A_d)
        w2B = cp.tile([128, 1024], F16); nc.sync.dma_start(w2B[:], w2B_d)
        w2sh = cp.tile([65, 128], F16); nc.sync.dma_start(w2sh[:], w2sh_d)
        w2sl = cp.tile([65, 128], F16); nc.sync.dma_start(w2sl[:], w2sl_d)
        w3r = cp.tile([128, 96], F16); nc.sync.dma_start(w3r[:], w3r_d)
        w3s = cp.tile([128, 12], F16); nc.sync.dma_start(w3s[:], w3s_d)
        b3c = cp.tile([12, 1], F32); nc.sync.dma_start(b3c[:], b3_d)
        diag = cp.tile([128, 128], F32); nc.sync.dma_start(diag[:], dg_d)
        ones64 = cp.tile([64, 1], F32); nc.gpsimd.memset(ones64[:], 1.0)
        ones128 = cp.tile([128, 1], F32); nc.gpsimd.memset(ones128[:], 1.0)
        ones2 = cp.tile([2, 128], F16); nc.gpsimd.memset(ones2[:], 1.0)

        # ---------------- stages --------------------------------------------
        def stL(s, d):
            d['p27h'] = bp.tile([28, 4096], F16, tag="p27h", name="p27h")
            nc.sync.dma_start(d['p27h'][:].rearrange("p (q n) -> p q n", q=4),
                              p27h_d[s].rearrange("q p n -> p q n"))
            d['p27l'] = bp.tile([28, 4096], F16, tag="p27l", name="p27l")
            nc.sync.dma_start(d['p27l'][:].rearrange("p (q n) -> p q n", q=4),
                              p27l_d[s].rearrange("q p n -> p q n"))

        def st0(s, d):
            p27h, p27l = d.pop('p27h'), d.pop('p27l')
            H = pp.tile([128, 1024], F32, tag="ps")
            for cch in range(2):
                for q in range(4):
                    o = 1024 * q + 512 * cch
                    hi = slice(o, o + 512)
                    osl = slice(512 * cch, 512 * cch + 512)
                    nc.tensor.matmul(H[:64, osl], c1h[:, 64 * q:64 * q + 64],
                                     p27h[:, hi], start=(q == 0), stop=False)
                    nc.tensor.matmul(H[:64, osl], c1h[:, 64 * q:64 * q + 64],
                                     p27l[:, hi], start=False, stop=False)
                    nc.tensor.matmul(H[:64, osl], c1l[:, 64 * q:64 * q + 64],
                                     p27h[:, hi], start=False, stop=(q == 3))
            m2l = sp.tile([65, 1024], F32, tag="m2l", bufs=1)
            nc.scalar.activation(m2l[:64, :], H[:64, :], AF.Relu)
            nc.gpsimd.memset(m2l[64:65, :], 1.0)
            m2h = sp.tile([65, 1024], F16, tag="m2h", bufs=3)
            nc.scalar.activation(m2h[:64, :], H[:64, :], AF.Relu)
            nc.gpsimd.memset(m2h[64:65, :], 1.0)
            msq = sp.tile([64, 1024], F32, tag="msq", bufs=1)
            nc.scalar.activation(msq[:], m2l[:64, :], AF.Square)
            nsqp = pp.tile([128, 1024], F32, tag="ps")
            for cch in range(2):
                nc.tensor.matmul(nsqp[:1, 512 * cch:512 * cch + 512], ones64[:],
                                 msq[:, 512 * cch:512 * cch + 512],
                                 start=True, stop=True)
            tap("m2l_%d" % s, m2l[:])
            # fp16 hi/lo dist operands: lhsT = [m2; ones] hi/lo,
            # rhs = [2*m2; -nsq] hi/lo (3-term split, lo*lo dropped)
            m2lo = sp.tile([65, 1024], F16, tag="m2lo", bufs=3)
            nc.vector.tensor_tensor(m2lo[:64, :], m2l[:64, :], m2h[:64, :],
                                    op=mybir.AluOpType.subtract)
            nc.gpsimd.memset(m2lo[64:65, :], 0.0)
            m2rh = sp.tile([65, 1024], F16, tag="m2rh", bufs=2)
            nc.scalar.activation(m2rh[:64, :], H[:64, :], AF.Relu, scale=2.0)
            nc.scalar.activation(m2rh[64:65, :], nsqp[:1, :], AF.Copy,
                                 scale=-1.0)
            m2rl = sp.tile([65, 1024], F16, tag="m2rl", bufs=2)
            nc.vector.scalar_tensor_tensor(
                m2rl[:64, :], m2l[:64, :], 2.0, m2rh[:64, :],
                op0=mybir.AluOpType.mult, op1=mybir.AluOpType.subtract)
            nqlo = sp.tile([1, 1024], F16, tag="nqlo", bufs=1)
            nc.vector.scalar_tensor_tensor(
                nqlo[:], nsqp[:1, :], -1.0, m2rh[64:65, :],
                op0=mybir.AluOpType.mult, op1=mybir.AluOpType.subtract)
            nc.sync.dma_start(m2rl[64:65, :], nqlo[:])
            mt2h = bp.tile([128, 8, 128], F16, tag="mt2h", bufs=2)
            nc.sync.dma_start(mt2h[:, :, :64], m2h[:64, :], transpose=True)
            nc.sync.dma_start(mt2h[:, :, 64:], m2lo[:64, :], transpose=True)
            tap("mt2h_%d" % s, mt2h[:].rearrange("p t c -> p (t c)"))
            mt2d = dp.tile([1024, 128], F16, tag="mt2d")
            nc.sync.dma_start(mt2d[:].rearrange("(t p) c -> p t c", p=128),
                              mt2h[:])
            d['m2h'], d['m2lo'] = m2h, m2lo
            d['m2rh'], d['m2rl'], d['mt2d'] = m2rh, m2rl, mt2d

        def st1(s, d):
            if stage < 2:
                return
            m2h, m2lo = d['m2h'], d['m2lo']
            m2rh, m2rl = d.pop('m2rh'), d.pop('m2rl')
            idxn = sp.tile([128, 8, 8], U16, tag="idxn", bufs=2)
            vals2 = sp.tile([128, 8, 8], F32, tag="vals2", bufs=2)
            mt2d = d.pop('mt2d')
            # g2 slot = th*4096 + kk*512 + tl*128 + p: half th gathers as soon
            # as top-k for its 4 token tiles is done; y2 chunk th consumes it
            g2 = bp.tile([128, 2, 8, 4, 128], F16, tag="g2", name="g2")
            for t in range(8):
                A = pp.tile([128, 1024], F32, tag="psA")
                for cch in range(2):
                    sl = slice(512 * cch, 512 * cch + 512)
                    tsl = slice(128 * t, 128 * t + 128)
                    nc.tensor.matmul(A[:, sl], m2h[:, tsl], m2rh[:, sl],
                                     start=True, stop=False)
                    nc.tensor.matmul(A[:, sl], m2h[:, tsl], m2rl[:, sl],
                                     start=False, stop=False)
                    nc.tensor.matmul(A[:, sl], m2lo[:, tsl], m2rh[:, sl],
                                     start=False, stop=True)
                Af = sp.tile([128, 1024], F32, tag="af", bufs=2)
                nc.scalar.activation(Af[:], A[:], AF.Copy)
                nc.vector.tensor_add(Af[:, 128 * t:128 * t + 128],
                                     Af[:, 128 * t:128 * t + 128], diag[:])
                nc.vector.max(vals2[:, t, :], Af[:])
                nc.vector.max_index(idxn[:, t, :], vals2[:, t, :], Af[:])
                if stage >= 3 and t % 4 == 3:
                    th = t // 4
                    idxv2 = sp.tile([128, 32], U16, tag="idxv2", bufs=2,
                                    name="idxv2")
                    nc.vector.tensor_copy(
                        idxv2[:].rearrange("p (kk tl) -> p kk tl", kk=8),
                        idxn[:, 4 * th:4 * th + 4, :].rearrange(
                            "p tl kk -> p kk tl"))
                    w2idx = _emit_wrapped_idx(nc, dp, sp, idxv2[:],
                                              "i2h%d" % th)
                    nc.gpsimd.dma_gather(
                        g2[:, th].rearrange("p kk tl n -> p (kk tl n)"
                                            ).rearrange("p (o n) -> p o n", o=1),
                        mt2d[:],
                        w2idx[:].bitcast(I16), 4096, 4096, 128,
                        transpose=True, single_packet=False)
            tap("idxn_%d" % s, idxn[:])
            if stage < 3:
                return
            tap("g2_%d" % s, g2[:].rearrange("p a b c d -> p (a b c d)"))
            d['g2'] = g2

        def st2(s, d):
            if stage < 4:
                return
            m2h, m2lo, g2 = d.pop('m2h'), d.pop('m2lo'), d.pop('g2')
            y2 = pp.tile([128, 1024], F32, tag="ps")
            for cch in range(2):
                sl = slice(512 * cch, 512 * cch + 512)
                for kk in range(8):
                    ksl = slice(128 * kk, 128 * kk + 128)
                    rhs = g2[:, cch, kk].rearrange("p tl n -> p (tl n)")
                    nc.tensor.matmul(y2[:, sl], w2A[:, ksl],
                                     rhs, start=(kk == 0), stop=False)
                    nc.tensor.matmul(y2[:, sl], w2B[:, ksl],
                                     rhs, start=False, stop=False)
                nc.tensor.matmul(y2[:, sl], w2sh[:], m2h[:, sl],
                                 start=False, stop=False)
                nc.tensor.matmul(y2[:, sl], w2sl[:], m2h[:, sl],
                                 start=False, stop=False)
                nc.tensor.matmul(y2[:, sl], w2sh[:], m2lo[:, sl],
                                 start=False, stop=True)
            m3l = sp.tile([128, 1024], F32, tag="m3l", bufs=3)
            nc.scalar.activation(m3l[:], y2[:], AF.Relu)
            tap("m3_%d" % s, m3l[:])
            m3h = sp.tile([128, 1024], F16, tag="m3h", bufs=3)
            nc.scalar.activation(m3h[:], y2[:], AF.Relu)
            m3lo = sp.tile([128, 1024], F16, tag="m3lo", bufs=2)
            nc.vector.tensor_tensor(m3lo[:], m3l[:], m3h[:],
                                    op=mybir.AluOpType.subtract)
            if stage < 4.3:
                return
            msq3 = sp.tile([128, 1024], F32, tag="msq3", bufs=1)
            nc.scalar.activation(msq3[:], m3l[:], AF.Square)
            nsq3p = pp.tile([128, 1024], F32, tag="ps")
            for cch in range(2):
                nc.tensor.matmul(nsq3p[:1, 512 * cch:512 * cch + 512], ones128[:],
                                 msq3[:, 512 * cch:512 * cch + 512],
                                 start=True, stop=True)
            nsq3n = sp.tile([1, 1024], F32, tag="nsq3", bufs=1)
            nc.scalar.activation(nsq3n[:], nsq3p[:1, :], AF.Copy, scale=-0.5)
            # fp16 hi/lo pair of -nsq3 for the rank-1 -nsq_j term
            nsq3hl = sp.tile([2, 1024], F16, tag="nsq3hl", bufs=2)
            nc.scalar.activation(nsq3hl[0:1, :], nsq3n[:], AF.Copy)
            nsq3lo = sp.tile([1, 1024], F16, tag="n3lo", bufs=1)
            nc.vector.tensor_tensor(nsq3lo[:], nsq3n[:], nsq3hl[0:1, :],
                                    op=mybir.AluOpType.subtract)
            nc.sync.dma_start(nsq3hl[1:2, :], nsq3lo[:])
            if stage < 4.6:
                return
            mt3h = bp.tile([128, 8, 128], F16, tag="mt3h", bufs=2)
            nc.sync.dma_start(mt3h[:], m3h[:], transpose=True)
            mt3d = dp.tile([1024, 128], F16, tag="mt3d")
            nc.sync.dma_start(mt3d[:].rearrange("(t p) c -> p t c", p=128),
                              mt3h[:])
            d['m3l'], d['m3h'], d['m3lo'] = m3l, m3h, m3lo
            d['nsq3hl'], d['mt3d'] = nsq3hl, mt3d

        def st3(s, d):
            if stage < 5:
                return
            m3h3, m3lo3 = d['m3h'], d.pop('m3lo')
            nsq3hl = d.pop('nsq3hl')
            idx3 = sp.tile([128, 8, 8], U16, tag="idx3", bufs=2)
            vals3 = sp.tile([128, 8, 8], F32, tag="vals3", bufs=2)
            mt3d = d.pop('mt3d')
            g3 = bp.tile([128, 2, 8, 4, 128], F16, tag="g3", name="g3")
            for t in range(8):
                A3 = pp.tile([128, 1024], F32, tag="psA")
                for cch in range(2):
                    sl = slice(512 * cch, 512 * cch + 512)
                    tsl = slice(128 * t, 128 * t + 128)
                    nc.tensor.matmul(A3[:, sl], m3h3[:, tsl], m3h3[:, sl],
                                     start=True, stop=False)
                    nc.tensor.matmul(A3[:, sl], m3h3[:, tsl], m3lo3[:, sl],
                                     start=False, stop=False)
                    nc.tensor.matmul(A3[:, sl], m3lo3[:, tsl], m3h3[:, sl],
                                     start=False, stop=False)
                    nc.tensor.matmul(A3[:, sl], ones2[:],
                                     nsq3hl[:, sl], start=False, stop=True)
                Af3 = sp.tile([128, 1024], F32, tag="af", bufs=2)
                nc.scalar.activation(Af3[:], A3[:], AF.Copy)
                nc.vector.tensor_add(Af3[:, 128 * t:128 * t + 128],
                                     Af3[:, 128 * t:128 * t + 128], diag[:])
                nc.vector.max(vals3[:, t, :], Af3[:])
                nc.vector.max_index(idx3[:, t, :], vals3[:, t, :], Af3[:])
                if stage >= 6 and t % 4 == 3:
                    th = t // 4
                    idxv3 = sp.tile([128, 32], U16, tag="idxv3", bufs=2,
                                    name="idxv3")
                    nc.vector.tensor_copy(
                        idxv3[:].rearrange("p (kk tl) -> p kk tl", kk=8),
                        idx3[:, 4 * th:4 * th + 4, :].rearrange(
                            "p tl kk -> p kk tl"))
                    w3idx = _emit_wrapped_idx(nc, dp, sp, idxv3[:],
                                              "i3h%d" % th)
                    nc.gpsimd.dma_gather(
                        g3[:, th].rearrange("p kk tl n -> p (kk tl n)"
                                            ).rearrange("p (o n) -> p o n", o=1),
                        mt3d[:],
                        w3idx[:].bitcast(I16), 4096, 4096, 128,
                        transpose=True, single_packet=False)
            tap("idx3_%d" % s, idx3[:])
            if stage < 6:
                return
            tap("g3_%d" % s, g3[:].rearrange("p a b c d -> p (a b c d)"))
            d['g3'] = g3

        def st4(s, d):
            if stage < 7:
                return
            m3h, g3 = d.pop('m3h'), d.pop('g3')
            d.pop('m3l')
            y3p = pp.tile([128, 1024], F32, tag="ps")
            for cch in range(2):
                sl = slice(512 * cch, 512 * cch + 512)
                for kk in range(8):
                    nc.tensor.matmul(y3p[:12, sl],
                                     w3r[:, 12 * kk:12 * kk + 12],
                                     g3[:, cch, kk].rearrange(
                                         "p tl n -> p (tl n)"),
                                     start=(kk == 0), stop=False)
                nc.tensor.matmul(y3p[:12, sl], w3s[:], m3h[:, sl],
                                 start=False, stop=True)
            y3sb = sp.tile([12, 1024], F32, tag="y3", bufs=1)
            nc.scalar.activation(y3sb[:], y3p[:12, :], AF.Identity, bias=b3c[:])
            tap("y3_%d" % s, y3sb[:])
            # contiguous store; pixel_shuffle happens on the host
            nc.sync.dma_start(out_d[s], y3sb[:])

        # ---------------- software-pipelined wave emission ------------------
        seq = [ss for _ in range(repeat) for ss in range(samples)]
        n = len(seq)
        state = [dict() for _ in range(n)]
        stages = [stL, st0, st1, st2, st3, st4]
        depth = len(stages)
        BLK = 2   # samples pipelined together (HW-validated depth)
        for b in range(0, n, BLK):
            hi = min(b + BLK, n)
            for w in range(b, hi + depth - 1):
                if b <= w < hi:
                    stL(seq[w], state[w])
                for k in range(depth - 1, 0, -1):
                    i = w - k
                    if b <= i < hi:
                        stages[k](seq[i], state[i])

    return nc


_CACHE = {}


def _get_compiled():
    if 'nc' not in _CACHE:
        nc = bacc.Bacc("TRN2", target_bir_lowering=False, debug=False,
                       num_devices=N_CORES, num_swdge_queues=4)
        build_program(nc, SAMPLES)
        nc.compile()
        _CACHE['nc'] = nc
    return _CACHE['nc']


def make_in_maps(x, consts):
    in_maps = []
    for c in range(N_CORES):
        shard = np.ascontiguousarray(x[c * SAMPLES:(c + 1) * SAMPLES],
                                     dtype=np.float32)
        m = dict(consts)
        m['p27hi'], m['p27lo'] = build_p27(shard)
        in_maps.append(m)
    return in_maps


def kernel(x, conv1_w, conv1_b, conv2_w, conv2_b, conv3_w, conv3_b, **_ignored):
    x = np.asarray(x, np.float32)
    consts = build_consts(conv1_w, conv1_b, conv2_w, conv2_b, conv3_w, conv3_b)
    nc = _get_compiled()
    in_maps = make_in_maps(x, consts)
    res = bass_utils.run_bass_kernel_spmd(nc, in_maps, core_ids=list(range(N_CORES)))
    y3 = np.concatenate([res.results[c]['out'] for c in range(N_CORES)], axis=0)
    return shuffle_out(y3)


def shuffle_out(y3):
    """y3 (B, 12, 1024) with channel rows co' = q*3+ch -> (B, 3, 64, 64)."""
    B = y3.shape[0]
    y = y3.reshape(B, 4, 3, 32, 32)                # [b][q=(sy,sx)][ch][h][w]
    out = np.zeros((B, 3, 64, 64), np.float32)
    for q in range(4):
        sy, sx = q >> 1, q & 1
        out[:, :, sy::2, sx::2] = y[:, q]
    return out.astype(np.float32)


if __name__ == '__main__':
    nc = _get_compiled()
    print("compiled ok")
